# revision 1
# baseline (speedup 1.0000x reference)
"""Expert-parallel BruteForce MoE kernel for 8 TRN2 NeuronCores.

Model: N=1024 tokens, D=512 d_model, H=2048 d_hidden, E=8 experts, top-K=2.
  logits = inp @ gate_w.T + gate_b ; top2 -> softmax scores
  y(tok,e) = gelu(x @ w1[e].T + b1[e]) @ w2[e].T + b2[e]
  out = LN( sum_k score_k * y(tok, e_k) )

Strategy (exact, static shapes): core e owns expert e. Every core computes
the gate for all tokens and derives G[:, e] = per-token weight of expert e
(softmax score if e is in that token's top-2, else 0). Core e then computes
Z_e = G[:, e] * (gelu(X @ w1[e].T + b1[e]) @ w2[e].T + b2[e]) for ALL 1024
tokens, dense.  sum_e Z_e equals the routed-and-combined MoE output.

Pipelining: tokens are processed in two halves. For each half: layer-1 (all
16 h-chunks) -> layer-2 -> gate-scale -> ReduceScatter -> LayerNorm + store
of that half's 64-token shard. The first half's RS+LN overlap the second
half's compute. Host reassembles the shards.

DMA: the gate weights, b1 (pre-transposed to [128,16]) and the first token
half are packed into ONE DRAM tensor ("xg") so the critical head is a
single issue+transfer; w1T is split into two k-halves with alternating
accumulation start order so layer-1 begins as soon as the first half lands.

Matmul dtype float32r: byte-compatible with f32, 4x PE rate vs plain f32
(1 cycle/row for moving dim >= 256) at ~tf32 accuracy (2e-4 rel err e2e).
The gate consumes the same bytes via f32 bitcast views so top-2 selection
matches the reference exactly. gelu is computed as 0.5*t*(1+erf(t/sqrt2))
so the whole kernel uses one ACT table set (erf+sigmoid); LN rsqrt is
Newton on DVE (no sqrt table).
"""

import numpy as np

import concourse.bass as bass
import concourse.bacc as bacc
import concourse.tile as tile
from concourse import mybir
from concourse import bass_utils

E, D, H, K, N = 8, 512, 2048, 2, 1024
P = 128
EPS = 1e-5
NEG_BIG = -1e30
RSQRT2 = 0.7071067811865476

KC = D // P      # 4  contraction chunks over d_model
HC = H // P      # 16 chunks over d_hidden
TC = N // P      # 8  token chunks of 128
TW = 512         # tokens per pipeline half (= moving width for layer-1)
NTW = N // TW    # 2
TCH = TW // P    # 4  token chunks per half
SH = TW // E     # 64: tokens per core per RS half

F32 = mybir.dt.float32
F32R = mybir.dt.float32r

XOFF = E + HC            # 24: xg cols = [gwT(8) | b1p(16) | x half0 (512)]
XGW = XOFF + TW          # 536
# aux layout: [b2(512), lnw(512), lnb(512), gb(8), sel(8)]
AUXN = 3 * D + 2 * E


def _chunked(dram, kc, p=P):
    """AP view of a [kc*P, M] DRAM tensor as [P, kc, M] (partition-major)."""
    m = dram.shape[1]
    return bass.AP(tensor=dram[:, :].tensor, offset=0,
                   ap=[[m, p], [p * m, kc], [1, m]])


def _bcast(ap, p=P):
    """AP that reads `ap` (a 1-D DRAM view) replicated across p partitions."""
    return bass.AP(tensor=ap.tensor, offset=ap.offset, ap=[[0, p]] + list(ap.ap))


def build_nc(mm_dtype=F32R, single_core=False):
    """Build the SPMD program (same on all 8 cores; per-core data differs).

    single_core=True replaces the collectives with local DMAs so TimelineSim
    (single-core, no collectives) can time the kernel; numerics differ.
    """
    nc = bacc.Bacc("TRN2", target_bir_lowering=False, debug=False,
                   num_devices=1 if single_core else E)
    MM = mm_dtype

    # ---- per-core external inputs ----
    xg = nc.dram_tensor("xg", [D, XGW], MM, kind="ExternalInput")   # packed head
    xTb = nc.dram_tensor("xTb", [D, TW], MM, kind="ExternalInput")  # x half1
    w1T = nc.dram_tensor("w1T", [D, H], MM, kind="ExternalInput")   # w1[e].T
    w2T = nc.dram_tensor("w2T", [H, D], MM, kind="ExternalInput")   # w2[e].T
    aux = nc.dram_tensor("aux", [AUXN], F32, kind="ExternalInput")  # packed vectors
    # rows 0:64 = tokens [c*64, (c+1)*64), rows 64:128 = [512+c*64, 512+(c+1)*64)
    out = nc.dram_tensor("out", [P, D], F32, kind="ExternalOutput")

    # internal DRAM for the chunked collective (separate tensors so the
    # first RS only depends on the first half's writes)
    zdr = [nc.dram_tensor(f"zdram{i}", [TW, D], F32) for i in range(NTW)]
    zrd = [nc.dram_tensor(f"zred{i}", [SH, D], F32) for i in range(NTW)]

    with tile.TileContext(nc) as tc:
        with (
            tc.tile_pool(name="persist", bufs=1) as persist,
            tc.tile_pool(name="work", bufs=4) as work,
            tc.tile_pool(name="zout", bufs=3) as zout,
            tc.tile_pool(name="psg", bufs=1, space="PSUM") as psg,
            tc.tile_pool(name="ps1", bufs=5, space="PSUM") as ps1,
            tc.tile_pool(name="ps2", bufs=2, space="PSUM") as ps2,
        ):
            xf = (lambda ap: ap.bitcast(F32)) if MM == F32R else (lambda ap: ap)

            # ---- persistent SBUF loads, ordered by first use ----
            xg_sb = persist.tile([P, KC, XGW], MM, tag="xg")
            xg_view = _chunked(xg, KC)
            w1T_sb = persist.tile([P, KC, H], MM, tag="w1T")
            w1T_view = _chunked(w1T, KC)
            for k in range(KC):
                nc.sync.dma_start(out=xg_sb[:, k:k + 1, :], in_=xg_view[:, k:k + 1, :])
                nc.sync.dma_start(out=w1T_sb[:, k:k + 1, :], in_=w1T_view[:, k:k + 1, :])

            xTb_sb = persist.tile([P, KC, TW], MM, tag="xTb")
            nc.sync.dma_start(out=xTb_sb, in_=_chunked(xTb, KC))

            w2T_sb = persist.tile([P, HC, D], MM, tag="w2T")
            w2T_view = _chunked(w2T, HC)
            HH = HC // 2
            nc.sync.dma_start(out=w2T_sb[:, 0:HH, :], in_=w2T_view[:, 0:HH, :])
            aux_sb = persist.tile([P, AUXN], F32, tag="aux")
            nc.sync.dma_start(out=aux_sb, in_=_bcast(aux[:]))
            nc.sync.dma_start(out=w2T_sb[:, HH:HC, :], in_=w2T_view[:, HH:HC, :])
            b2_sb = aux_sb[:, 0:D]
            lnw_sb = aux_sb[:, D:2 * D]
            lnb_sb = aux_sb[:, 2 * D:3 * D]
            gb_sb = aux_sb[:, 3 * D:3 * D + E]
            sel_sb = aux_sb[:, 3 * D + E:3 * D + 2 * E]

            eps_sb = persist.tile([P, 1], F32, tag="eps")
            nc.vector.memset(eps_sb, EPS)
            # first ACT op: pulls the single erf/sigmoid table in early
            warm = persist.tile([P, 1], F32, tag="warm")
            nc.scalar.activation(warm, eps_sb, mybir.ActivationFunctionType.Erf)

            # b1 views from the packed xg (chunk 0, cols 8:24) + b1/sqrt2
            b1_sb = xf(xg_sb[:, 0, E:E + HC])                  # [P, 16]
            b1h_sb = persist.tile([P, HC], F32, tag="b1h")
            nc.vector.tensor_scalar(
                out=b1h_sb, in0=b1_sb, scalar1=RSQRT2, scalar2=None,
                op0=mybir.AluOpType.mult,
            )

            def xcol(t):
                """lhsT view of token chunk t for the gate, per k."""
                if t < TCH:
                    return lambda k: xf(
                        xg_sb[:, k, XOFF + t * P:XOFF + (t + 1) * P])
                return lambda k: xf(
                    xTb_sb[:, k, (t - TCH) * P:(t - TCH + 1) * P])

            # ---- gate matmuls: logits for all tokens (full f32) ----
            La = persist.tile([P, TC, E], F32, tag="La")
            for t in range(TC):
                pg = psg.tile([P, E], F32, tag="psg")
                col = xcol(t)
                for k in range(KC):
                    nc.tensor.matmul(
                        pg,
                        lhsT=col(k),
                        rhs=xf(xg_sb[:, k, 0:E]),
                        start=(k == 0),
                        stop=(k == KC - 1),
                    )
                nc.vector.tensor_copy(out=La[:, t, :], in_=pg)

            def layer1(tw, g1):
                rhs_of = (lambda k: xg_sb[:, k, XOFF:XOFF + TW]) if tw == 0 \
                    else (lambda k: xTb_sb[:, k, :])
                for h in range(HC):
                    p1 = ps1.tile([P, TW], F32, tag="ps1")
                    for j, k in enumerate(range(KC)):
                        nc.tensor.matmul(
                            p1,
                            lhsT=w1T_sb[:, k, h * P:(h + 1) * P],
                            rhs=rhs_of(k),
                            start=(j == 0),
                            stop=(j == KC - 1),
                        )
                    # gelu(t) = 0.5*(t)*(1+erf(t/sqrt2)), t = p1 + b1
                    er = work.tile([P, TW], F32, tag="er")
                    nc.scalar.activation(
                        er, p1, mybir.ActivationFunctionType.Erf,
                        bias=b1h_sb[:, h:h + 1], scale=RSQRT2,
                    )
                    ht = work.tile([P, TW], F32, tag="ht")
                    nc.vector.tensor_scalar(
                        out=ht, in0=p1, scalar1=b1_sb[:, h:h + 1], scalar2=0.5,
                        op0=mybir.AluOpType.add, op1=mybir.AluOpType.mult,
                    )
                    nc.vector.scalar_tensor_tensor(
                        out=g1[:, h, :], in0=er, scalar=1.0, in1=ht,
                        op0=mybir.AluOpType.add, op1=mybir.AluOpType.mult,
                    )

            def gate_chain():
                # top-2 mask math on [P, TC, E]; emitted after the first
                # layer-1 half so the ACT sigmoid never blocks gelu evictions
                X = mybir.AxisListType.X
                nc.vector.tensor_tensor(
                    out=La, in0=La,
                    in1=gb_sb[:, None, :].to_broadcast((P, TC, E)),
                    op=mybir.AluOpType.add,
                )
                v1 = work.tile([P, TC], F32, tag="v1")
                nc.vector.reduce_max(out=v1, in_=La, axis=X)
                eq1 = work.tile([P, TC, E], F32, tag="eq1")
                nc.vector.tensor_tensor(
                    out=eq1, in0=La, in1=v1[:, :, None].to_broadcast((P, TC, E)),
                    op=mybir.AluOpType.is_equal,
                )
                Lm = work.tile([P, TC, E], F32, tag="Lm")
                nc.vector.scalar_tensor_tensor(
                    out=Lm, in0=eq1, scalar=NEG_BIG, in1=La,
                    op0=mybir.AluOpType.mult, op1=mybir.AluOpType.add,
                )
                v2 = work.tile([P, TC], F32, tag="v2")
                nc.vector.reduce_max(out=v2, in_=Lm, axis=X)
                eq2 = work.tile([P, TC, E], F32, tag="eq2")
                nc.vector.tensor_tensor(
                    out=eq2, in0=Lm, in1=v2[:, :, None].to_broadcast((P, TC, E)),
                    op=mybir.AluOpType.is_equal,
                )
                s2 = work.tile([P, TC], F32, tag="s2")
                nc.vector.tensor_sub(s2, v2, v1)
                nc.scalar.activation(s2, s2, mybir.ActivationFunctionType.Sigmoid)
                s1 = work.tile([P, TC], F32, tag="s1")
                nc.vector.tensor_scalar(
                    out=s1, in0=s2, scalar1=-1.0, scalar2=1.0,
                    op0=mybir.AluOpType.mult, op1=mybir.AluOpType.add,
                )
                A1 = work.tile([P, TC, E], F32, tag="A1")
                nc.vector.tensor_mul(
                    A1, eq1, s1[:, :, None].to_broadcast((P, TC, E)))
                A2 = work.tile([P, TC, E], F32, tag="A2")
                nc.vector.tensor_mul(
                    A2, eq2, s2[:, :, None].to_broadcast((P, TC, E)))
                nc.vector.tensor_add(A1, A1, A2)
                nc.vector.tensor_mul(
                    A1, A1, sel_sb[:, None, :].to_broadcast((P, TC, E)))
                gcol = persist.tile([P, TC], F32, tag="gcol")
                nc.vector.reduce_sum(out=gcol, in_=A1, axis=X)
                return gcol

            def layer2(tw, g1, gcol):
                for tl in range(TCH):
                    t = tw * TCH + tl
                    p2 = ps2.tile([P, D], F32, tag="ps2")
                    for h in range(HC):
                        nc.tensor.matmul(
                            p2,
                            lhsT=g1[:, h, tl * P:(tl + 1) * P],
                            rhs=w2T_sb[:, h, :],
                            start=(h == 0),
                            stop=(h == HC - 1),
                        )
                    zt = zout.tile([P, D], F32, tag="zt")
                    nc.vector.tensor_add(zt, p2, b2_sb)
                    nc.vector.tensor_scalar(
                        out=zt, in0=zt, scalar1=gcol[:, t:t + 1], scalar2=None,
                        op0=mybir.AluOpType.mult,
                    )
                    nc.sync.dma_start(
                        out=zdr[tw][tl * P:(tl + 1) * P, :], in_=zt)
                if not single_core:
                    nc.gpsimd.collective_compute(
                        "ReduceScatter",
                        mybir.AluOpType.add,
                        replica_groups=[list(range(E))],
                        ins=[zdr[tw][:, :].opt()],
                        outs=[zrd[tw][:, :].opt()],
                    )

            zsb = persist.tile([P, D], F32, tag="zsb")

            def ln_half(half):
                """LayerNorm + store of this half's 64-token shard."""
                o = half * SH
                src = zdr[half][0:SH, :] if single_core else zrd[half][:, :]
                nc.sync.dma_start(out=zsb[o:o + SH, :], in_=src)
                z = zsb[o:o + SH, :]
                stats = work.tile([P, 6], F32, tag="stats")
                nc.vector.bn_stats(out=stats[0:SH], in_=z)
                mv = work.tile([P, 2], F32, tag="mv")
                nc.vector.bn_aggr(out=mv[0:SH], in_=stats[0:SH])
                # rstd via bit-hack + 3 Newton steps (no sqrt table needed)
                rstd = work.tile([P, 1], F32, tag="rstd")
                ve = work.tile([P, 1], F32, tag="ve")
                nc.vector.tensor_scalar(
                    out=ve[0:SH], in0=mv[0:SH, 1:2], scalar1=float(EPS),
                    scalar2=None, op0=mybir.AluOpType.add,
                )
                I32 = mybir.dt.int32
                nc.vector.tensor_scalar(
                    out=rstd[0:SH].bitcast(I32), in0=ve[0:SH].bitcast(I32),
                    scalar1=1, scalar2=None,
                    op0=mybir.AluOpType.arith_shift_right,
                )
                nc.vector.tensor_scalar(
                    out=rstd[0:SH].bitcast(I32), in0=rstd[0:SH].bitcast(I32),
                    scalar1=-1, scalar2=0x5F3759DF,
                    op0=mybir.AluOpType.mult, op1=mybir.AluOpType.add,
                )
                t1 = work.tile([P, 1], F32, tag="t1")
                for _ in range(3):        # y *= 1.5 - 0.5*v*y*y
                    nc.vector.tensor_mul(t1[0:SH], rstd[0:SH], rstd[0:SH])
                    nc.vector.tensor_mul(t1[0:SH], t1[0:SH], ve[0:SH])
                    nc.vector.tensor_scalar(
                        out=t1[0:SH], in0=t1[0:SH], scalar1=-0.5, scalar2=1.5,
                        op0=mybir.AluOpType.mult, op1=mybir.AluOpType.add,
                    )
                    nc.vector.tensor_mul(rstd[0:SH], rstd[0:SH], t1[0:SH])
                xn = work.tile([P, D], F32, tag="xn")
                nc.vector.tensor_scalar(
                    out=xn[0:SH], in0=z, scalar1=mv[0:SH, 0:1],
                    scalar2=rstd[0:SH],
                    op0=mybir.AluOpType.subtract, op1=mybir.AluOpType.mult,
                )
                nc.vector.tensor_mul(xn[0:SH], xn[0:SH], lnw_sb[0:SH])
                nc.vector.tensor_add(xn[0:SH], xn[0:SH], lnb_sb[0:SH])
                nc.sync.dma_start(out=out[o:o + SH, :], in_=xn[0:SH])

            # ---- pipelined halves ----
            g1a = persist.tile([P, HC, TW], MM, tag="g1a")
            g1b = persist.tile([P, HC, TW], MM, tag="g1b")
            layer1(0, g1a)
            gcol = gate_chain()
            layer2(0, g1a, gcol)
            layer1(1, g1b)
            ln_half(0)
            layer2(1, g1b, gcol)
            ln_half(1)

    nc.compile()
    return nc


_CACHE = {}


def _get_nc(key, mm_dtype):
    if key not in _CACHE:
        _CACHE[key] = build_nc(mm_dtype)
    return _CACHE[key]


MM_DTYPE = "f32r"  # "f32" | "f32r"


def make_in_maps(inputs, mm_np=np.float32):
    inp = np.asarray(inputs["inp"], dtype=np.float32)
    gate_w = np.asarray(inputs["gate_w"], dtype=np.float32)
    gate_b = np.asarray(inputs["gate_b"], dtype=np.float32)
    w1 = np.asarray(inputs["w1"], dtype=np.float32)
    b1 = np.asarray(inputs["b1"], dtype=np.float32)
    w2 = np.asarray(inputs["w2"], dtype=np.float32)
    b2 = np.asarray(inputs["b2"], dtype=np.float32)
    ln_w = np.asarray(inputs["ln_w"], dtype=np.float32)
    ln_b = np.asarray(inputs["ln_b"], dtype=np.float32)

    xT = np.ascontiguousarray(inp.T)                      # [D, N]
    gwT = np.ascontiguousarray(gate_w.T)                  # [D, E]
    eye = np.eye(E, dtype=np.float32)

    in_maps = []
    for c in range(E):
        xgv = np.zeros((D, XGW), np.float32)
        xgv[:, 0:E] = gwT
        # b1 pre-transposed into chunk 0: b1p[p, h] = b1[c][h*128+p]
        xgv[0:P, E:XOFF] = b1[c].reshape(HC, P).T
        xgv[:, XOFF:XGW] = xT[:, 0:TW]
        auxv = np.concatenate([b2[c], ln_w, ln_b, gate_b, eye[c]]).astype(np.float32)
        in_maps.append({
            "xg": xgv.astype(mm_np),
            "xTb": np.ascontiguousarray(xT[:, TW:N]).astype(mm_np),
            "w1T": np.ascontiguousarray(w1[c].T).astype(mm_np),   # [D, H]
            "w2T": np.ascontiguousarray(w2[c].T).astype(mm_np),   # [H, D]
            "aux": auxv,
        })
    return in_maps


def kernel(**inputs):
    mm_dt = F32R if MM_DTYPE == "f32r" else F32
    nc = _get_nc(MM_DTYPE, mm_dt)
    in_maps = make_in_maps(inputs)
    res = bass_utils.run_bass_kernel_spmd(nc, in_maps, core_ids=list(range(E)))
    # core c's output rows 0:64 are tokens [c*64,(c+1)*64); rows 64:128 are
    # tokens [512+c*64, 512+(c+1)*64)
    full = np.empty((N, D), np.float32)
    for c in range(E):
        o = res.results[c]["out"]
        full[c * SH:(c + 1) * SH] = o[0:SH]
        full[TW + c * SH:TW + (c + 1) * SH] = o[SH:P]
    return full



# revision 2
# speedup vs baseline: 1.7249x; 1.7249x over previous
"""Routed expert-parallel BruteForce MoE kernel for 8 TRN2 NeuronCores.

Model: N=1024 tokens, D=512 d_model, H=2048 d_hidden, E=8 experts, top-K=2.
  logits = inp @ gate_w.T + gate_b ; top2 -> softmax scores
  y(tok,e) = gelu(x @ w1[e].T + b1[e]) @ w2[e].T + b2[e]
  out = LN( sum_k score_k * y(tok, e_k) )

Strategy: core e owns expert e. The HOST computes the gate logits only to
make the ROUTING decision (which tokens hit expert e); each core receives
just its ~C routed tokens packed [D, C] (C = max per-expert count, padded).
The DEVICE recomputes the gate for its slots in exact f32 (matching the
host's top-2 choice; min 2nd-vs-3rd logit gap is ~2e-4 >> f32 noise) and
derives the softmax score of its own expert exactly like the dense kernel.
FFN runs in bf16 (full PE rate, half the HBM bytes): gelu via the ACT
Gelu table with b1 as per-partition bias; layer-2 psum gets +b2 then is
scaled by the slot's gate score during the bf16 eviction.

Combine: per-expert outputs [C, D] bf16 are AllGathered to [8C, D]; core c
owns tokens [128c, 128c+128) and pulls its 2 scaled contributions per token
with a SWDGE dma_gather (idx computed on host, int16, wrapped in 16
partitions and replicated x8), adds them, LayerNorms, and stores its
128-token shard. Host reassembles the 8 shards.

PE p-state: a short chain of junk warm-up matmuls keeps PE busy during the
initial weight DMA so the real matmuls run at the ramped 2.4 GHz clock.
"""

import numpy as np
import ml_dtypes

import concourse.bass as bass
import concourse.bacc as bacc
import concourse.tile as tile
from concourse import mybir
from concourse import bass_utils
from concourse import library_config

E, D, H, K, N = 8, 512, 2048, 2, 1024
P = 128
KC = D // P      # 4  contraction chunks over d_model
HC = H // P      # 16 chunks over d_hidden
EPS = 1e-5
NEG_BIG = -1e30

F32 = mybir.dt.float32
BF16 = mybir.dt.bfloat16
I16 = mybir.dt.int16

GW = E + HC                  # 24: xg cols = [gwT(8) | b1p(16) | Xp(C)]
# aux layout: [b2(512), lnw(512), lnb(512), gb(8), sel(8)]
AUXN = 3 * D + 2 * E
NWARM = 7                    # junk matmuls to ramp the PE p-state


def _chunked(dram, kc, p=P):
    """AP view of a [kc*P, M] DRAM tensor as [P, kc, M] (partition-major)."""
    m = dram.shape[1]
    return bass.AP(tensor=dram[:, :].tensor, offset=0,
                   ap=[[m, p], [p * m, kc], [1, m]])


def _bcast(ap, p=P):
    """AP that reads `ap` (a 1-D DRAM view) replicated across p partitions."""
    return bass.AP(tensor=ap.tensor, offset=ap.offset, ap=[[0, p]] + list(ap.ap))


def build_nc(C, single_core=False):
    """Build the SPMD program for slot capacity C (multiple of 64).

    single_core=True replaces the AllGather with a local DMA so TimelineSim
    (single-core, no collectives) can time the kernel; numerics differ.
    """
    NCC = (C + P - 1) // P   # slot chunks
    nc = bacc.Bacc("TRN2", target_bir_lowering=False, debug=False,
                   num_devices=1 if single_core else E)

    xg = nc.dram_tensor("xg", [D, GW + C], F32, kind="ExternalInput")
    w1b = nc.dram_tensor("w1b", [D, H], BF16, kind="ExternalInput")   # w1[e].T
    w2b = nc.dram_tensor("w2b", [H, D], BF16, kind="ExternalInput")   # w2[e].T
    aux = nc.dram_tensor("aux", [AUXN], F32, kind="ExternalInput")
    idx = nc.dram_tensor("idx", [P, 16], I16, kind="ExternalInput")
    out = nc.dram_tensor("out", [P, D], F32, kind="ExternalOutput")

    ybuf = nc.dram_tensor("ybuf", [C, D], BF16)
    agbuf = nc.dram_tensor("agbuf", [E * C, D], BF16)

    with tile.TileContext(nc) as tc:
        with (
            tc.tile_pool(name="persist", bufs=1) as persist,
            tc.tile_pool(name="work", bufs=4) as work,
            tc.tile_pool(name="yout", bufs=3) as yout,
            tc.tile_pool(name="psw", bufs=1, space="PSUM") as psw,
            tc.tile_pool(name="psg", bufs=2, space="PSUM") as psg,
            tc.tile_pool(name="ps1", bufs=3, space="PSUM") as ps1,
            tc.tile_pool(name="ps2", bufs=2, space="PSUM") as ps2,
        ):
            nc.gpsimd.load_library(library_config.mlp)

            # ---- tiny loads first (cheap, unblock gather prep + gate) ----
            idx_sb = persist.tile([P, 16], I16, tag="idx")
            nc.sync.dma_start(out=idx_sb, in_=idx[:, :])
            aux_sb = persist.tile([P, AUXN], F32, tag="aux")
            nc.sync.dma_start(out=aux_sb, in_=_bcast(aux[:]))
            b2_sb = aux_sb[:, 0:D]
            lnw_sb = aux_sb[:, D:2 * D]
            lnb_sb = aux_sb[:, 2 * D:3 * D]
            gb_sb = aux_sb[:, 3 * D:3 * D + E]
            sel_sb = aux_sb[:, 3 * D + E:3 * D + 2 * E]

            # ---- warm-up junk matmuls: ramp PE while DMAs stream in ----
            jl = persist.tile([P, P], BF16, tag="jl")
            nc.vector.memset(jl, 0.0)
            jw = persist.tile([P, D], BF16, tag="jw")
            nc.vector.memset(jw, 0.0)
            for w in range(NWARM):
                pw = psw.tile([P, D], F32, tag="psw")
                nc.tensor.matmul(pw, lhsT=jl, rhs=jw, start=True, stop=True)

            # ---- xg: [gwT | b1p | Xp] f32, chunk-pipelined ----
            xg_sb = persist.tile([P, KC, GW + C], F32, tag="xg")
            xg_view = _chunked(xg, KC)
            xbf = persist.tile([P, KC, C], BF16, tag="xbf")
            for k in range(KC):
                nc.sync.dma_start(out=xg_sb[:, k:k + 1, :],
                                  in_=xg_view[:, k:k + 1, :])
                nc.vector.tensor_copy(out=xbf[:, k, :],
                                      in_=xg_sb[:, k, GW:GW + C])

            # w1 first half so layer-1 can start early, then w2, then w1 rest
            w1b_sb = persist.tile([P, KC, H], BF16, tag="w1b")
            w1b_view = _chunked(w1b, KC)
            w2b_sb = persist.tile([P, HC, D], BF16, tag="w2b")
            w2b_view = _chunked(w2b, HC)
            for k in range(KC):
                nc.sync.dma_start(out=w1b_sb[:, k:k + 1, 0:H // 2],
                                  in_=w1b_view[:, k:k + 1, 0:H // 2])
            for k in range(KC):
                nc.sync.dma_start(out=w1b_sb[:, k:k + 1, H // 2:H],
                                  in_=w1b_view[:, k:k + 1, H // 2:H])
            for hh in range(4):
                nc.sync.dma_start(out=w2b_sb[:, 4 * hh:4 * hh + 4, :],
                                  in_=w2b_view[:, 4 * hh:4 * hh + 4, :])

            b1p_sb = xg_sb[:, 0, E:E + HC]                  # [P, 16] f32

            # ---- gate: exact-f32 logits for all C slots ----
            La = persist.tile([P, NCC, E], F32, tag="La")
            for cc in range(NCC):
                cw = min(P, C - cc * P)
                pg = psg.tile([P, E], F32, tag="psg")
                for k in range(KC):
                    nc.tensor.matmul(
                        pg[0:cw],
                        lhsT=xg_sb[:, k, GW + cc * P:GW + cc * P + cw],
                        rhs=xg_sb[:, k, 0:E],
                        start=(k == 0),
                        stop=(k == KC - 1),
                    )
                nc.vector.tensor_copy(out=La[:, cc, :], in_=pg)

            def gate_chain():
                """Per-slot gate score of OWN expert, [P, NCC] f32."""
                X = mybir.AxisListType.X
                nc.vector.tensor_tensor(
                    out=La, in0=La,
                    in1=gb_sb[:, None, :].to_broadcast((P, NCC, E)),
                    op=mybir.AluOpType.add,
                )
                v1 = work.tile([P, NCC], F32, tag="v1")
                nc.vector.reduce_max(out=v1, in_=La, axis=X)
                eq1 = work.tile([P, NCC, E], F32, tag="eq1")
                nc.vector.tensor_tensor(
                    out=eq1, in0=La, in1=v1[:, :, None].to_broadcast((P, NCC, E)),
                    op=mybir.AluOpType.is_equal,
                )
                Lm = work.tile([P, NCC, E], F32, tag="Lm")
                nc.vector.scalar_tensor_tensor(
                    out=Lm, in0=eq1, scalar=NEG_BIG, in1=La,
                    op0=mybir.AluOpType.mult, op1=mybir.AluOpType.add,
                )
                v2 = work.tile([P, NCC], F32, tag="v2")
                nc.vector.reduce_max(out=v2, in_=Lm, axis=X)
                eq2 = work.tile([P, NCC, E], F32, tag="eq2")
                nc.vector.tensor_tensor(
                    out=eq2, in0=Lm, in1=v2[:, :, None].to_broadcast((P, NCC, E)),
                    op=mybir.AluOpType.is_equal,
                )
                s2 = work.tile([P, NCC], F32, tag="s2")
                nc.vector.tensor_sub(s2, v2, v1)
                nc.scalar.activation(s2, s2, mybir.ActivationFunctionType.Sigmoid)
                s1 = work.tile([P, NCC], F32, tag="s1")
                nc.vector.tensor_scalar(
                    out=s1, in0=s2, scalar1=-1.0, scalar2=1.0,
                    op0=mybir.AluOpType.mult, op1=mybir.AluOpType.add,
                )
                A1 = work.tile([P, NCC, E], F32, tag="A1")
                nc.vector.tensor_mul(
                    A1, eq1, s1[:, :, None].to_broadcast((P, NCC, E)))
                A2 = work.tile([P, NCC, E], F32, tag="A2")
                nc.vector.tensor_mul(
                    A2, eq2, s2[:, :, None].to_broadcast((P, NCC, E)))
                nc.vector.tensor_add(A1, A1, A2)
                nc.vector.tensor_mul(
                    A1, A1, sel_sb[:, None, :].to_broadcast((P, NCC, E)))
                gcol = persist.tile([P, NCC], F32, tag="gcol")
                nc.vector.reduce_sum(out=gcol, in_=A1, axis=X)
                return gcol

            gcol = gate_chain()

            # ---- layer 1: g1 = gelu(x @ w1.T + b1), bf16 [P, HC, C] ----
            g1 = persist.tile([P, HC, C], BF16, tag="g1")
            for h in range(HC):
                p1 = ps1.tile([P, C], F32, tag="ps1")
                for k in range(KC):
                    nc.tensor.matmul(
                        p1,
                        lhsT=w1b_sb[:, k, h * P:(h + 1) * P],
                        rhs=xbf[:, k, :],
                        start=(k == 0),
                        stop=(k == KC - 1),
                    )
                nc.scalar.activation(
                    g1[:, h, :], p1, mybir.ActivationFunctionType.Gelu,
                    bias=b1p_sb[:, h:h + 1], scale=1.0,
                )

            # ---- layer 2 + scaled bf16 eviction to ybuf ----
            for cc in range(NCC):
                cw = min(P, C - cc * P)
                p2 = ps2.tile([P, D], F32, tag="ps2")
                for h in range(HC):
                    nc.tensor.matmul(
                        p2[0:cw],
                        lhsT=g1[:, h, cc * P:cc * P + cw],
                        rhs=w2b_sb[:, h, :],
                        start=(h == 0),
                        stop=(h == HC - 1),
                    )
                zb = yout.tile([P, D], F32, tag="zb")
                nc.vector.tensor_add(zb[0:cw], p2[0:cw], b2_sb[0:cw])
                yb = yout.tile([P, D], BF16, tag="yb")
                nc.scalar.activation(
                    yb[0:cw], zb[0:cw], mybir.ActivationFunctionType.Copy,
                    scale=gcol[0:cw, cc:cc + 1],
                )
                nc.sync.dma_start(out=ybuf[cc * P:cc * P + cw, :],
                                  in_=yb[0:cw])

            # ---- exchange + owner-side combine ----
            if single_core:
                nc.sync.dma_start(out=agbuf[0:C, :], in_=ybuf[:, :])
            else:
                nc.gpsimd.collective_compute(
                    "AllGather",
                    mybir.AluOpType.bypass,
                    replica_groups=[list(range(E))],
                    ins=[ybuf[:, :].opt()],
                    outs=[agbuf[:, :].opt()],
                )
            g2 = persist.tile([P, 2, D], BF16, tag="g2")
            nc.gpsimd.dma_gather(
                out_ap=g2, in_ap=agbuf[:, :], idxs_ap=idx_sb[:, :],
                num_idxs=2 * P, num_idxs_reg=2 * P, elem_size=D,
            )
            z = persist.tile([P, D], F32, tag="z")
            nc.vector.tensor_add(z, g2[:, 0, :], g2[:, 1, :])

            # ---- LayerNorm + store ----
            stats = work.tile([P, 6], F32, tag="stats")
            nc.vector.bn_stats(out=stats, in_=z)
            mv = work.tile([P, 2], F32, tag="mv")
            nc.vector.bn_aggr(out=mv, in_=stats)
            # rstd via bit-hack + 3 Newton steps (no sqrt table needed)
            rstd = work.tile([P, 1], F32, tag="rstd")
            ve = work.tile([P, 1], F32, tag="ve")
            nc.vector.tensor_scalar(
                out=ve, in0=mv[:, 1:2], scalar1=float(EPS),
                scalar2=None, op0=mybir.AluOpType.add,
            )
            I32 = mybir.dt.int32
            nc.vector.tensor_scalar(
                out=rstd.bitcast(I32), in0=ve.bitcast(I32),
                scalar1=1, scalar2=None,
                op0=mybir.AluOpType.arith_shift_right,
            )
            nc.vector.tensor_scalar(
                out=rstd.bitcast(I32), in0=rstd.bitcast(I32),
                scalar1=-1, scalar2=0x5F3759DF,
                op0=mybir.AluOpType.mult, op1=mybir.AluOpType.add,
            )
            t1 = work.tile([P, 1], F32, tag="t1")
            for _ in range(3):        # y *= 1.5 - 0.5*v*y*y
                nc.vector.tensor_mul(t1, rstd, rstd)
                nc.vector.tensor_mul(t1, t1, ve)
                nc.vector.tensor_scalar(
                    out=t1, in0=t1, scalar1=-0.5, scalar2=1.5,
                    op0=mybir.AluOpType.mult, op1=mybir.AluOpType.add,
                )
                nc.vector.tensor_mul(rstd, rstd, t1)
            xn = work.tile([P, D], F32, tag="xn")
            nc.vector.tensor_scalar(
                out=xn, in0=z, scalar1=mv[:, 0:1], scalar2=rstd,
                op0=mybir.AluOpType.subtract, op1=mybir.AluOpType.mult,
            )
            nc.vector.tensor_mul(xn, xn, lnw_sb)
            nc.vector.tensor_add(xn, xn, lnb_sb)
            nc.sync.dma_start(out=out[:, :], in_=xn)

    nc.compile()
    return nc


_CACHE = {}


def _get_nc(C):
    if C not in _CACHE:
        _CACHE[C] = build_nc(C)
    return _CACHE[C]


def route(inp, gate_w, gate_b):
    """Host-side routing DECISION (top-2 expert ids per token); all scoring
    arithmetic is recomputed on-device in exact f32."""
    logits = inp.astype(np.float32) @ gate_w.T.astype(np.float32) + gate_b
    top2 = np.argsort(-logits, axis=1, kind="stable")[:, :K]   # [N, 2]
    return top2


def make_in_maps(inputs, C=None):
    inp = np.asarray(inputs["inp"], dtype=np.float32)
    gate_w = np.asarray(inputs["gate_w"], dtype=np.float32)
    gate_b = np.asarray(inputs["gate_b"], dtype=np.float32)
    w1 = np.asarray(inputs["w1"], dtype=np.float32)
    b1 = np.asarray(inputs["b1"], dtype=np.float32)
    w2 = np.asarray(inputs["w2"], dtype=np.float32)
    b2 = np.asarray(inputs["b2"], dtype=np.float32)
    ln_w = np.asarray(inputs["ln_w"], dtype=np.float32)
    ln_b = np.asarray(inputs["ln_b"], dtype=np.float32)

    top2 = route(inp, gate_w, gate_b)
    toks = [np.where((top2[:, 0] == e) | (top2[:, 1] == e))[0] for e in range(E)]
    maxc = max(len(t) for t in toks)
    if C is None:
        C = max(((maxc + 63) // 64) * 64, P)
    assert maxc <= C

    slot_of = np.full((E, N), -1, np.int64)
    for e in range(E):
        slot_of[e, toks[e]] = np.arange(len(toks[e]))

    xT = np.ascontiguousarray(inp.T)                      # [D, N]
    gwT = np.ascontiguousarray(gate_w.T)                  # [D, E]
    eye = np.eye(E, dtype=np.float32)

    in_maps = []
    for c in range(E):
        xgv = np.zeros((D, GW + C), np.float32)
        xgv[:, 0:E] = gwT
        # b1 pre-transposed into chunk 0: b1p[p, h] = b1[c][h*128+p]
        xgv[0:P, E:GW] = b1[c].reshape(HC, P).T
        xgv[:, GW:GW + len(toks[c])] = xT[:, toks[c]]
        auxv = np.concatenate(
            [b2[c], ln_w, ln_b, gate_b, eye[c]]).astype(np.float32)
        # gather rows for owned tokens [128c, 128c+128): contribution k of
        # token t lives at row top2[t][k]*C + slot_of[top2[t][k], t]
        own = np.arange(P * c, P * (c + 1))
        rows = np.empty(2 * P, np.int64)
        for kk in range(K):
            ee = top2[own, kk]
            rows[kk * P:(kk + 1) * P] = ee * C + slot_of[ee, own]
        blk = np.zeros((16, 16), np.int16)
        blk[np.arange(2 * P) % 16, np.arange(2 * P) // 16] = \
            rows.astype(np.int16)
        in_maps.append({
            "xg": xgv,
            "w1b": np.ascontiguousarray(w1[c].T).astype(ml_dtypes.bfloat16),
            "w2b": np.ascontiguousarray(w2[c].T).astype(ml_dtypes.bfloat16),
            "aux": auxv,
            "idx": np.tile(blk, (E, 1)),
        })
    return in_maps, C


def kernel(**inputs):
    in_maps, C = make_in_maps(inputs)
    nc = _get_nc(C)
    res = bass_utils.run_bass_kernel_spmd(nc, in_maps, core_ids=list(range(E)))
    full = np.empty((N, D), np.float32)
    for c in range(E):
        full[P * c:P * (c + 1)] = res.results[c]["out"]
    return full


# revision 3
# speedup vs baseline: 2.0217x; 1.1721x over previous
"""Routed expert-parallel BruteForce MoE kernel for 8 TRN2 NeuronCores.

Model: N=1024 tokens, D=512 d_model, H=2048 d_hidden, E=8 experts, top-K=2.
  logits = inp @ gate_w.T + gate_b ; top2 -> softmax scores
  y(tok,e) = gelu(x @ w1[e].T + b1[e]) @ w2[e].T + b2[e]
  out = LN( sum_k score_k * y(tok, e_k) )

Strategy: core e owns expert e. The HOST computes the gate logits only to
make the ROUTING decision (which tokens hit expert e); each core receives
just its ~C routed tokens packed [D, C] (C = max per-expert count, padded).
The DEVICE recomputes the gate for its slots in exact f32 (matching the
host's top-2 choice; min 2nd-vs-3rd logit gap is ~2e-4 >> f32 noise) and
derives the softmax score of its own expert like the dense kernel -- the
gate weight COLUMNS are permuted per core so the own expert is column 0,
making the score selection a static slice. gate_b and b2 enter their
matmuls as an extra ones-row contraction step (no SBUF broadcasts).

FFN runs in bf16 (full PE rate, half the HBM bytes): gelu via the ACT Gelu
table with b1 as per-partition bias; layer-2 psum (+b2 row) is scaled by
the slot's gate score during the bf16 ACT-Copy eviction.

Combine: per-expert outputs [C, D] bf16 are AllGathered to [8C, D]; core c
owns tokens [128c, 128c+128) and pulls its 2 scaled contributions per token
with a SWDGE dma_gather (idx computed on host, int16, wrapped in 16
partitions and replicated x8), adds them, LayerNorms, and stores its
128-token shard. Host reassembles the 8 shards.

PE p-state: a short chain of junk warm-up matmuls keeps PE busy during the
initial DMA so the real matmuls run at the ramped 2.4 GHz clock.
"""

import numpy as np
import ml_dtypes

import concourse.bass as bass
import concourse.bacc as bacc
import concourse.tile as tile
from concourse import mybir
from concourse import bass_utils
from concourse import library_config

E, D, H, K, N = 8, 512, 2048, 2, 1024
P = 128
KC = D // P      # 4  contraction chunks over d_model
HC = H // P      # 16 chunks over d_hidden
EPS = 1e-5
NEG_BIG = -1e30

F32 = mybir.dt.float32
BF16 = mybir.dt.bfloat16
I16 = mybir.dt.int16

GW = E + HC                  # 24: xg cols = [gwT(8, permuted) | b1p(16) | Xp]
NWARM = 6                    # junk matmuls to ramp the PE p-state


def _chunked(dram, kc, p=P):
    """AP view of a [kc*P, M] DRAM tensor as [P, kc, M] (partition-major)."""
    m = dram.shape[1]
    return bass.AP(tensor=dram[:, :].tensor, offset=0,
                   ap=[[m, p], [p * m, kc], [1, m]])


def _bcast(ap, p=P):
    """AP that reads `ap` (a 1-D DRAM view) replicated across p partitions."""
    return bass.AP(tensor=ap.tensor, offset=ap.offset, ap=[[0, p]] + list(ap.ap))


def build_nc(C, single_core=False):
    """Build the SPMD program for slot capacity C (multiple of 32).

    single_core=True drops the AllGather (gather reads ybuf directly) so
    TimelineSim (single-core, no collectives) can time the kernel; numerics
    differ.
    """
    NCC = (C + P - 1) // P   # slot chunks
    nc = bacc.Bacc("TRN2", target_bir_lowering=False, debug=False,
                   num_devices=1 if single_core else E)

    xg = nc.dram_tensor("xg", [D, GW + C], F32, kind="ExternalInput")
    w1b = nc.dram_tensor("w1b", [D, H], BF16, kind="ExternalInput")  # w1[e].T
    # w2b rows: [w2[e].T (H) | b2 row + zero pad (P)]
    w2b = nc.dram_tensor("w2b", [H + P, D], BF16, kind="ExternalInput")
    gbr = nc.dram_tensor("gbr", [1, E], F32, kind="ExternalInput")  # permuted
    lwb = nc.dram_tensor("lwb", [2 * D], F32, kind="ExternalInput")
    idx = nc.dram_tensor("idx", [P, 16], I16, kind="ExternalInput")
    out = nc.dram_tensor("out", [P, D], F32, kind="ExternalOutput")

    ybuf = nc.dram_tensor("ybuf", [C, D], BF16)
    agbuf = nc.dram_tensor("agbuf", [E * C, D], BF16)

    with tile.TileContext(nc) as tc:
        with (
            tc.tile_pool(name="persist", bufs=1) as persist,
            tc.tile_pool(name="work", bufs=4) as work,
            tc.tile_pool(name="yout", bufs=3) as yout,
            tc.tile_pool(name="psw", bufs=1, space="PSUM") as psw,
            tc.tile_pool(name="psg", bufs=1, space="PSUM") as psg,
            tc.tile_pool(name="ps1", bufs=3, space="PSUM") as ps1,
            tc.tile_pool(name="ps2", bufs=2, space="PSUM") as ps2,
        ):
            # ---- xg first: gate + layer-1 critical path ----
            xg_sb = persist.tile([P, KC, GW + C], F32, tag="xg")
            xg_view = _chunked(xg, KC)
            xbf = persist.tile([P, KC, C], BF16, tag="xbf")
            for k2 in range(2):
                nc.sync.dma_start(out=xg_sb[:, 2 * k2:2 * k2 + 2, :],
                                  in_=xg_view[:, 2 * k2:2 * k2 + 2, :])
                nc.vector.tensor_copy(out=xbf[:, 2 * k2:2 * k2 + 2, :],
                                      in_=xg_sb[:, 2 * k2:2 * k2 + 2, GW:GW + C])
            gbr_sb = persist.tile([P, E], F32, tag="gbr")
            nc.sync.dma_start(out=gbr_sb[0:1, :], in_=gbr[:, :])

            # ---- warm-up junk matmuls: ramp PE while DMAs stream in ----
            jl = persist.tile([P, P], BF16, tag="jl")
            nc.vector.memset(jl, 0.0)
            jw = persist.tile([P, D], BF16, tag="jw")
            nc.vector.memset(jw, 0.0)
            for w in range(NWARM):
                pw = psw.tile([P, D], F32, tag="psw")
                nc.tensor.matmul(pw, lhsT=jl, rhs=jw, start=True, stop=True)

            # ones rows for the bias-row matmul trick
            ones_f = persist.tile([P, C], F32, tag="ones_f")
            nc.vector.memset(ones_f[0:1, :], 1.0)
            ones_b = persist.tile([P, C], BF16, tag="ones_b")
            nc.vector.memset(ones_b[0:1, :], 1.0)

            # ---- weights, ordered by first use ----
            w1b_sb = persist.tile([P, KC, H], BF16, tag="w1b")
            w1b_view = _chunked(w1b, KC)
            w2b_sb = persist.tile([P, HC + 1, D], BF16, tag="w2b")
            w2b_view = _chunked(w2b, HC + 1)
            for k in range(KC):
                nc.sync.dma_start(out=w1b_sb[:, k:k + 1, 0:H // 2],
                                  in_=w1b_view[:, k:k + 1, 0:H // 2])
            for k in range(KC):
                nc.sync.dma_start(out=w1b_sb[:, k:k + 1, H // 2:H],
                                  in_=w1b_view[:, k:k + 1, H // 2:H])
            lwb_sb = persist.tile([P, 2 * D], F32, tag="lwb")
            nc.sync.dma_start(out=lwb_sb, in_=_bcast(lwb[:]))
            nc.sync.dma_start(out=w2b_sb[:, 16:17, :], in_=w2b_view[:, 16:17, :])
            for hh in range(4):
                nc.sync.dma_start(out=w2b_sb[:, 4 * hh:4 * hh + 4, :],
                                  in_=w2b_view[:, 4 * hh:4 * hh + 4, :])
            idx_sb = persist.tile([P, 16], I16, tag="idx")
            nc.sync.dma_start(out=idx_sb, in_=idx[:, :])

            lnw_sb = lwb_sb[:, 0:D]
            lnb_sb = lwb_sb[:, D:2 * D]
            b1p_sb = xg_sb[:, 0, E:E + HC]                  # [P, 16] f32

            # ---- gate: exact-f32 logits for all C slots (col 0 = own) ----
            La = persist.tile([P, NCC, E], F32, tag="La")
            for cc in range(NCC):
                cw = min(P, C - cc * P)
                pg = psg.tile([P, E], F32, tag="psg")
                for k in range(KC):
                    nc.tensor.matmul(
                        pg[0:cw],
                        lhsT=xg_sb[:, k, GW + cc * P:GW + cc * P + cw],
                        rhs=xg_sb[:, k, 0:E],
                        start=(k == 0),
                        stop=False,
                    )
                nc.tensor.matmul(
                    pg[0:cw],
                    lhsT=ones_f[0:1, cc * P:cc * P + cw],
                    rhs=gbr_sb[0:1, :],
                    start=False,
                    stop=True,
                )
                nc.vector.tensor_copy(out=La[:, cc, :], in_=pg)

            def gate_chain():
                """Per-slot gate score of OWN expert (column 0), [P,NCC]."""
                X = mybir.AxisListType.X
                v1 = work.tile([P, NCC], F32, tag="v1")
                nc.vector.reduce_max(out=v1, in_=La, axis=X)
                eq1 = work.tile([P, NCC, E], F32, tag="eq1")
                nc.vector.tensor_tensor(
                    out=eq1, in0=La, in1=v1[:, :, None].to_broadcast((P, NCC, E)),
                    op=mybir.AluOpType.is_equal,
                )
                Lm = work.tile([P, NCC, E], F32, tag="Lm")
                nc.vector.scalar_tensor_tensor(
                    out=Lm, in0=eq1, scalar=NEG_BIG, in1=La,
                    op0=mybir.AluOpType.mult, op1=mybir.AluOpType.add,
                )
                v2 = work.tile([P, NCC], F32, tag="v2")
                nc.vector.reduce_max(out=v2, in_=Lm, axis=X)
                s2 = work.tile([P, NCC], F32, tag="s2")
                nc.vector.tensor_sub(s2, v2, v1)
                nc.scalar.activation(s2, s2, mybir.ActivationFunctionType.Sigmoid)
                e2s = work.tile([P, NCC], F32, tag="e2s")
                nc.vector.tensor_tensor(
                    out=e2s, in0=Lm[:, :, 0], in1=v2,
                    op=mybir.AluOpType.is_equal,
                )
                nc.vector.tensor_mul(e2s, e2s, s2)          # e2 * s2
                s1 = work.tile([P, NCC], F32, tag="s1")
                nc.vector.tensor_scalar(
                    out=s1, in0=s2, scalar1=-1.0, scalar2=1.0,
                    op0=mybir.AluOpType.mult, op1=mybir.AluOpType.add,
                )
                nc.vector.tensor_mul(s1, s1, eq1[:, :, 0])  # e1 * s1
                gcol = persist.tile([P, NCC], F32, tag="gcol")
                nc.vector.tensor_add(gcol, s1, e2s)
                return gcol

            gcol = gate_chain()

            # ---- layer 1: g1 = gelu(x @ w1.T + b1), bf16 [P, HC, C] ----
            g1 = persist.tile([P, HC, C], BF16, tag="g1")
            for h in range(HC):
                p1 = ps1.tile([P, C], F32, tag="ps1")
                for k in range(KC):
                    nc.tensor.matmul(
                        p1,
                        lhsT=w1b_sb[:, k, h * P:(h + 1) * P],
                        rhs=xbf[:, k, :],
                        start=(k == 0),
                        stop=(k == KC - 1),
                    )
                nc.scalar.activation(
                    g1[:, h, :], p1, mybir.ActivationFunctionType.Gelu,
                    bias=b1p_sb[:, h:h + 1], scale=1.0,
                )

            # ---- layer 2 (+b2 ones-row) + scaled bf16 eviction ----
            for cc in range(NCC):
                cw = min(P, C - cc * P)
                p2 = ps2.tile([P, D], F32, tag="ps2")
                for h in range(HC):
                    nc.tensor.matmul(
                        p2[0:cw],
                        lhsT=g1[:, h, cc * P:cc * P + cw],
                        rhs=w2b_sb[:, h, :],
                        start=(h == 0),
                        stop=False,
                    )
                nc.tensor.matmul(
                    p2[0:cw],
                    lhsT=ones_b[0:1, cc * P:cc * P + cw],
                    rhs=w2b_sb[0:1, 16, :],
                    start=False,
                    stop=True,
                )
                yb = yout.tile([P, D], BF16, tag="yb")
                nc.scalar.activation(
                    yb[0:cw], p2[0:cw], mybir.ActivationFunctionType.Copy,
                    scale=gcol[0:cw, cc:cc + 1],
                )
                nc.sync.dma_start(out=ybuf[cc * P:cc * P + cw, :],
                                  in_=yb[0:cw])

            # ---- exchange + owner-side combine ----
            nc.gpsimd.load_library(library_config.mlp)
            if single_core:
                gsrc = ybuf
            else:
                nc.gpsimd.collective_compute(
                    "AllGather",
                    mybir.AluOpType.bypass,
                    replica_groups=[list(range(E))],
                    ins=[ybuf[:, :].opt()],
                    outs=[agbuf[:, :].opt()],
                )
                gsrc = agbuf
            g2 = persist.tile([P, 2, D], BF16, tag="g2")
            nc.gpsimd.dma_gather(
                out_ap=g2, in_ap=gsrc[:, :], idxs_ap=idx_sb[:, :],
                num_idxs=2 * P, num_idxs_reg=2 * P, elem_size=D,
            )
            z = persist.tile([P, D], F32, tag="z")
            nc.vector.tensor_add(z, g2[:, 0, :], g2[:, 1, :])

            # ---- LayerNorm + store ----
            stats = work.tile([P, 6], F32, tag="stats")
            nc.vector.bn_stats(out=stats, in_=z)
            mv = work.tile([P, 2], F32, tag="mv")
            nc.vector.bn_aggr(out=mv, in_=stats)
            # rstd via bit-hack + 2 Newton steps (no sqrt table needed)
            rstd = work.tile([P, 1], F32, tag="rstd")
            ve = work.tile([P, 1], F32, tag="ve")
            nc.vector.tensor_scalar(
                out=ve, in0=mv[:, 1:2], scalar1=float(EPS),
                scalar2=None, op0=mybir.AluOpType.add,
            )
            I32 = mybir.dt.int32
            nc.vector.tensor_scalar(
                out=rstd.bitcast(I32), in0=ve.bitcast(I32),
                scalar1=1, scalar2=None,
                op0=mybir.AluOpType.arith_shift_right,
            )
            nc.vector.tensor_scalar(
                out=rstd.bitcast(I32), in0=rstd.bitcast(I32),
                scalar1=-1, scalar2=0x5F3759DF,
                op0=mybir.AluOpType.mult, op1=mybir.AluOpType.add,
            )
            t1 = work.tile([P, 1], F32, tag="t1")
            for _ in range(2):        # y *= 1.5 - 0.5*v*y*y
                nc.vector.tensor_mul(t1, rstd, rstd)
                nc.vector.tensor_mul(t1, t1, ve)
                nc.vector.tensor_scalar(
                    out=t1, in0=t1, scalar1=-0.5, scalar2=1.5,
                    op0=mybir.AluOpType.mult, op1=mybir.AluOpType.add,
                )
                nc.vector.tensor_mul(rstd, rstd, t1)
            xn = work.tile([P, D], F32, tag="xn")
            nc.vector.tensor_scalar(
                out=xn, in0=z, scalar1=mv[:, 0:1], scalar2=rstd,
                op0=mybir.AluOpType.subtract, op1=mybir.AluOpType.mult,
            )
            nc.vector.tensor_mul(xn, xn, lnw_sb)
            nc.vector.tensor_add(xn, xn, lnb_sb)
            nc.sync.dma_start(out=out[:, :], in_=xn)

    nc.compile()
    return nc


_CACHE = {}


def _get_nc(C):
    if C not in _CACHE:
        _CACHE[C] = build_nc(C)
    return _CACHE[C]


def route(inp, gate_w, gate_b):
    """Host-side routing DECISION (top-2 expert ids per token); all scoring
    arithmetic is recomputed on-device in exact f32."""
    logits = inp.astype(np.float32) @ gate_w.T.astype(np.float32) + gate_b
    top2 = np.argsort(-logits, axis=1, kind="stable")[:, :K]   # [N, 2]
    return top2


def make_in_maps(inputs, C=None):
    inp = np.asarray(inputs["inp"], dtype=np.float32)
    gate_w = np.asarray(inputs["gate_w"], dtype=np.float32)
    gate_b = np.asarray(inputs["gate_b"], dtype=np.float32)
    w1 = np.asarray(inputs["w1"], dtype=np.float32)
    b1 = np.asarray(inputs["b1"], dtype=np.float32)
    w2 = np.asarray(inputs["w2"], dtype=np.float32)
    b2 = np.asarray(inputs["b2"], dtype=np.float32)
    ln_w = np.asarray(inputs["ln_w"], dtype=np.float32)
    ln_b = np.asarray(inputs["ln_b"], dtype=np.float32)

    top2 = route(inp, gate_w, gate_b)
    toks = [np.where((top2[:, 0] == e) | (top2[:, 1] == e))[0] for e in range(E)]
    maxc = max(len(t) for t in toks)
    if C is None:
        C = max(((maxc + 31) // 32) * 32, P)
    assert maxc <= C

    slot_of = np.full((E, N), -1, np.int64)
    for e in range(E):
        slot_of[e, toks[e]] = np.arange(len(toks[e]))

    xT = np.ascontiguousarray(inp.T)                      # [D, N]
    lwb = np.concatenate([ln_w, ln_b]).astype(np.float32)

    in_maps = []
    for c in range(E):
        # permute experts so own expert is column 0 (order invariant for
        # max/2nd-max); host and device then agree on "column 0 = own".
        perm = [c] + [e for e in range(E) if e != c]
        xgv = np.zeros((D, GW + C), np.float32)
        xgv[:, 0:E] = gate_w.T[:, perm]
        # b1 pre-transposed into chunk 0: b1p[p, h] = b1[c][h*128+p]
        xgv[0:P, E:GW] = b1[c].reshape(HC, P).T
        xgv[:, GW:GW + len(toks[c])] = xT[:, toks[c]]
        w2v = np.zeros((H + P, D), np.float32)
        w2v[0:H] = w2[c].T
        w2v[H] = b2[c]
        # gather rows for owned tokens [128c, 128c+128): contribution k of
        # token t lives at row top2[t][k]*C + slot_of[top2[t][k], t]
        own = np.arange(P * c, P * (c + 1))
        rows = np.empty(2 * P, np.int64)
        for kk in range(K):
            ee = top2[own, kk]
            rows[kk * P:(kk + 1) * P] = ee * C + slot_of[ee, own]
        blk = np.zeros((16, 16), np.int16)
        blk[np.arange(2 * P) % 16, np.arange(2 * P) // 16] = \
            rows.astype(np.int16)
        in_maps.append({
            "xg": xgv,
            "w1b": np.ascontiguousarray(w1[c].T).astype(ml_dtypes.bfloat16),
            "w2b": w2v.astype(ml_dtypes.bfloat16),
            "gbr": gate_b[perm].reshape(1, E).astype(np.float32),
            "lwb": lwb,
            "idx": np.tile(blk, (E, 1)),
        })
    return in_maps, C


def kernel(**inputs):
    in_maps, C = make_in_maps(inputs)
    nc = _get_nc(C)
    res = bass_utils.run_bass_kernel_spmd(nc, in_maps, core_ids=list(range(E)))
    full = np.empty((N, D), np.float32)
    for c in range(E):
        full[P * c:P * (c + 1)] = res.results[c]["out"]
    return full
